# revision 49
# baseline (speedup 1.0000x reference)
"""Trainium2 Bass kernel for BasisDecorrelationLoss.

Math: per sample b, with x = depth_basis[b] ([C=32, N=76800]) and mask m ([N]):
    mu_c  = (1/N) sum_n x[c,n]                      (unmasked spatial mean)
    S_cd  = sum_n x[c,n] x[d,n] m[n]                (masked Gram, the heavy part)
    t_c   = sum_n x[c,n] m[n]
    M     = sum_n m[n]
    cov   = (S - mu t^T - t mu^T + mu mu^T M) / M   (mean-centered masked covariance)
    zncc  = clamp(cov,eps) / (sigma sigma^T), loss_b = mean(zncc^2)
    loss  = mean_b loss_b

Device strategy (data-parallel, one sample per NeuronCore, 8 cores):
  The host folds the mask into the data: y = x * sqrt(m), rounded to fp8e4m3.
  Then S = Y Y^T is a *pure symmetric Gram* - no on-chip mask multiply, so
  the PE stream depends only on the DMA.  mu, t, M are host f64 sums (exact),
  and the zncc diagonal is exactly 1 by construction, so fp8 only perturbs
  the tiny off-diagonal terms (measured rel err ~1e-5 end to end).

  HBM layout is pre-transposed on the host to Y[p, macro, slot, c] with
  n = p*600 + u: each of the 128 partitions reads one contiguous 19.2KB run.
  The DMA is split into growing chunks on one queue (see CHUNKS_M) so the
  PE starts as soon as the first ~300KB lands and never waits again.

  The Gram runs as 150 fp8 DoubleRow matmuls.  Each one packs TWO u-pairs
  into a single LDWEIGHTS+MATMUL: lhsT = rhs = [128, 2, 64] where weight
  columns 0:32 belong to u-pair A = (u0, u1) and columns 32:64 to pair
  B = (u2, u3); the host stores each macro's 4 u-slabs in slot order
  [u0, u2, u1, u3] so one 3-dim AP ([part][ktile stride 64][free stride 1,
  64]) reads both pairs.  The [64, 64] PSUM output holds S_A in its top-left
  32x32 block and S_B bottom-right (off-diagonal blocks are cross-pair
  garbage that is simply never read).  K = 256 per instruction (DoubleRow
  sums two 128-partition k-tiles), out rows 64 = the full DR column
  capacity of the array (walrus emits col_grp 0xf, dst partition 0 - the
  only placement its DoubleRow codegen accepts).  Two alternating PSUM
  accumulators decouple consecutive steps; the host sums the 4 diagonal
  blocks and does the final covariance -> zncc math and the batch mean
  (the "scalar all-reduce").
"""

import ml_dtypes
import numpy as np

import concourse.bacc as bacc
import concourse.bass as bass
import concourse.tile as tile
from concourse import mybir
from concourse.bass_utils import run_bass_kernel_spmd

B = 8
C = 32
H, W = 240, 320
N = H * W            # 76800
P = 128              # SBUF partitions
NPP = N // P         # 600 u-values per partition
NMACRO = NPP // 4    # 150 macro-steps (4 u's each)
# macro-extents per DMA chunk, all on one HWDGE queue (strict FIFO
# completion; two queues race for the shared 16 DMA engines and stall
# unpredictably).  The PE's first reader of a chunk waits for the whole
# chunk and the DMA path has ~1.3us fixed latency + a bandwidth ramp, so
# the schedule front-loads a medium chunk (runway) and grows from there;
# sizes chosen to minimize max_k(arrival_k + PE time for remaining macros)
# against the measured arrival curve.
CHUNKS_M = [18, 14, 14, 16, 22, 30, 36]
EPS = 1e-10

_F32 = mybir.dt.float32
_F8 = mybir.dt.float8e4


def _build_kernel_body(tc: "tile.TileContext", y_d: bass.AP, out_d: bass.AP):
    nc = tc.nc

    with (
        tc.tile_pool(name="slab", bufs=1) as slab,
        tc.tile_pool(name="psum", bufs=1, space="PSUM") as psum,
        tc.tile_pool(name="outp", bufs=1) as outp,
    ):
        t_s = slab.tile([P, NMACRO, 4, C], _F8)
        # two alternating [64, 64] accumulators side by side in one PSUM tile
        accw = psum.tile([2 * C, 4 * C], _F32)

        # Warm the PE clock while waiting for the first DMA chunk: the PE
        # p-state ramps with continuous busy time, and it would otherwise
        # idle from the end of the preamble (~7us) until the first chunk
        # lands (~10us).  Same DR mode as the real stream (mode switches
        # flush the pipeline); zeroed scratch weights, junk accumulator
        # that is never read.
        dum = slab.tile([P, 4, C], _F8)
        junk = psum.tile([2 * C, 2 * C], _F32)
        nc.vector.memset(dum[:, :, :], 0.0)
        da = dum[:, 0, :]
        dap = bass.AP(tensor=da.tensor, offset=da.offset,
                      ap=[da.ap[0], [2 * C, 2], [1, 2 * C]])
        NDUM = 44
        for j in range(NDUM):
            nc.tensor.matmul(
                junk[:, :],
                lhsT=dap,
                rhs=dap,
                start=(j == 0),
                stop=(j == NDUM - 1),
                perf_mode=mybir.MatmulPerfMode.DoubleRow,
                tile_position=(0, 0),
            )

        m0 = 0
        for k, mc in enumerate(CHUNKS_M):
            nc.sync.dma_start(out=t_s[:, m0 : m0 + mc, :, :],
                              in_=y_d[:, m0 : m0 + mc, :, :])
            for i in range(m0, m0 + mc):
                base = t_s[:, i, 0, :]
                yp = bass.AP(tensor=base.tensor, offset=base.offset,
                             ap=[base.ap[0], [2 * C, 2], [1, 2 * C]])
                g = i % 2
                nc.tensor.matmul(
                    accw[:, 2 * C * g : 2 * C * (g + 1)],
                    lhsT=yp,
                    rhs=yp,
                    start=(i < 2),
                    stop=(i >= NMACRO - 2),
                    perf_mode=mybir.MatmulPerfMode.DoubleRow,
                    tile_position=(0, 0),
                )
            m0 += mc

        res = outp.tile([2 * C, 4 * C], _F32)
        nc.vector.tensor_copy(res, accw)
        nc.sync.dma_start(out=out_d, in_=res)


def _build_nc() -> bass.Bass:
    nc = bacc.Bacc()
    y = nc.declare_dram_parameter("y", [P, NMACRO, 4, C], _F8, isOutput=False)
    out = nc.declare_dram_parameter("out", [2 * C, 4 * C], _F32, isOutput=True)
    with tile.TileContext(nc) as tc:
        _build_kernel_body(tc, y[:], out[:])
    nc.finalize()
    return nc


def _finalize(gathered: list[np.ndarray], mu: np.ndarray, t: np.ndarray,
              M: np.ndarray) -> np.ndarray:
    """Per-sample [64, 128] block -> scalar loss, batch mean.

    Columns 0:64 = accumulator 0, columns 64:128 = accumulator 1; in each
    [64, 64] block the diagonal 32x32 sub-blocks are the pair-A / pair-B
    Gram sums.
    """
    total = 0.0
    for i, G in enumerate(gathered):
        G = G.astype(np.float64)
        S = (G[0:C, 0:C] + G[C : 2 * C, C : 2 * C]
             + G[0:C, 2 * C : 3 * C] + G[C : 2 * C, 3 * C : 4 * C])
        cov = (S - np.outer(mu[i], t[i]) - np.outer(t[i], mu[i])
               + np.outer(mu[i], mu[i]) * M[i]) / M[i]
        cov = np.maximum(cov, EPS)
        sig = np.sqrt(np.diag(cov))
        zncc = cov / np.outer(sig, sig)
        total += float(np.mean(zncc * zncc))
    return np.array(total / B, dtype=np.float32)


_NC_CACHE = None


def _run(depth_basis: np.ndarray, mask: np.ndarray, trace: bool = False):
    global _NC_CACHE
    if _NC_CACHE is None:
        _NC_CACHE = _build_nc()
    nc = _NC_CACHE

    x = np.asarray(depth_basis, dtype=np.float32).reshape(B, C, N)
    m = np.asarray(mask, dtype=np.float32).reshape(B, N)

    y8 = (x * np.sqrt(m)[:, None, :]).astype(ml_dtypes.float8_e4m3)
    # [B, C, N] with n = p*NPP + u  ->  [B, P, NMACRO, 4, C], u-slots
    # reordered [u0, u2, u1, u3] within each macro of 4.
    y5 = y8.reshape(B, C, P, NMACRO, 4)[:, :, :, :, [0, 2, 1, 3]]
    yh = np.ascontiguousarray(y5.transpose(0, 2, 3, 4, 1))

    x64 = x.astype(np.float64)
    m64 = m.astype(np.float64)
    mu = x64.mean(axis=2)                  # [B, C]
    t = np.einsum('bcn,bn->bc', x64, m64)  # [B, C]
    M = m64.sum(axis=1)                    # [B]

    in_maps = [{"y": yh[i]} for i in range(B)]
    r = run_bass_kernel_spmd(nc, in_maps, list(range(B)), trace=trace)
    gathered = [np.asarray(r.results[i]["out"]) for i in range(B)]
    return _finalize(gathered, mu, t, M), r


def kernel(depth_basis: np.ndarray, mask: np.ndarray) -> np.ndarray:
    loss, _ = _run(depth_basis, mask, trace=False)
    return loss
